# revision 1
# baseline (speedup 1.0000x reference)
"""Trainium2 Bass kernel for nn_LocalRNN (local GRU, chunked scan).

Problem: B=32, S=2048, I=H=256, ksize=16. Each ksize-chunk runs a GRU from
h0=0, so the 32*128=4096 chunks are independent length-16 GRU chains.

Sharding: data-parallel over chunks — core c gets batch rows [4c:4c+4],
i.e. 512 chains. Weights replicated.

Per-core kernel layout ("transposed"): gate/hidden dim on partitions, chain
(seq) index on the free dim. Per step t and seq-group g (2 groups x 256 seqs):

  gates[3H, seqs] = W_ih @ x_t^T + W_hh @ h_{t-1}^T     (PSUM accumulation)
  r = sigmoid(psum_r + (b_ih+b_hh)_r)                    (ScalarE, bias port)
  z = sigmoid(psum_z + (b_ih+b_hh)_z)
  n = tanh((psum_in + b_ih_n) + r*(psum_hn + b_hh_n))    (fused DVE stt ops)
  h = n + z*(h_prev - n)

The x-side and h-side matmuls for r/z accumulate into the same PSUM bank so
no explicit adds are needed; n keeps separate x/h banks because r multiplies
only the h side. PSUM budget: 4 banks per group x 2 groups = all 8 banks,
ping-ponged so one group's matmuls overlap the other group's elementwise.

Matmul operands and SBUF elementwise tensors are fp16 (PE fast-weight-load +
DVE 2x mode, ~8x finer mantissa than bf16; values are O(1) so fp16 range is
safe); PSUM accumulation is fp32. Host pre-transposes x / weights into
DMA-friendly contiguous blocks and inverts the output layout at the end.
"""

import sys

for _p in ("/opt/trn_rl_repo", "/root/.axon_site"):
    if _p not in sys.path:
        sys.path.insert(0, _p)

import ml_dtypes
import numpy as np

import concourse.bass as bass  # noqa: F401
import concourse.tile as tile
from concourse import bacc, mybir
from concourse.bass_utils import run_bass_kernel_spmd

# Problem constants (hardcoded per harness contract).
B, S, I, H = 32, 2048, 256, 256
KSIZE = 16
NCORES = 8
ROWS_PER_CORE = B // NCORES            # 4 batch rows per core
CHUNKS_PER_ROW = S // KSIZE            # 128
SEQS = ROWS_PER_CORE * CHUNKS_PER_ROW  # 512 chains per core
G = 2                                  # seq groups per core
NS = SEQS // G                         # 256 seqs per group
KT = 2                                 # contraction tiles (I/128 = H/128 = 2)

F32 = mybir.dt.float32
F16 = mybir.dt.float16
AF = mybir.ActivationFunctionType
OP = mybir.AluOpType

MM_DT = F16         # matmul operand + elementwise SBUF dtype
NP_MM_DT = np.float16


def build_nc():
    nc = bacc.Bacc("TRN2", target_bir_lowering=False, debug=False)

    # Inputs (host pre-transposed, contiguous per-DMA blocks).
    # xt[t, g, p, k, s] = x_shard[seq=g*NS+s, t, i=k*128+p]
    xt_d = nc.dram_tensor("xt", [KSIZE, G, 128, KT, NS], MM_DT, kind="ExternalInput")
    # wih_t[p, k, m] = W_ih[m, k*128+p]  (transposed weight, lhsT layout)
    wih_d = nc.dram_tensor("wih_t", [128, KT, 3 * H], MM_DT, kind="ExternalInput")
    whh_d = nc.dram_tensor("whh_t", [128, KT, 3 * H], MM_DT, kind="ExternalInput")
    # brz[p, mi] = (b_ih+b_hh)[mi*128+p] for mi in 0..3 (r0,r1,z0,z1)
    brz_d = nc.dram_tensor("brz", [128, 4], F32, kind="ExternalInput")
    # bhn[p, m] = b_hh[2H + m*128 + p]; bin[p, m] = b_ih[2H + m*128 + p]
    bhn_d = nc.dram_tensor("bhn", [128, 2], F32, kind="ExternalInput")
    bin_d = nc.dram_tensor("bin", [128, 2], F32, kind="ExternalInput")
    # out[t, g, p, m, s] = h_t[seq=g*NS+s, hdim=m*128+p]
    out_d = nc.dram_tensor("out", [KSIZE, G, 128, 2, NS], MM_DT, kind="ExternalOutput")

    with tile.TileContext(nc) as tc:
        with (
            tc.tile_pool(name="consts", bufs=1) as consts,
            tc.tile_pool(name="xp", bufs=8) as xp,
            tc.tile_pool(name="ps", bufs=2, space="PSUM") as ps,
            tc.tile_pool(name="work", bufs=4) as work,
            tc.tile_pool(name="hp", bufs=4) as hp,
        ):
            wih = consts.tile([128, KT, 3 * H], MM_DT)
            nc.sync.dma_start(wih[:], wih_d.ap())
            whh = consts.tile([128, KT, 3 * H], MM_DT)
            nc.sync.dma_start(whh[:], whh_d.ap())
            brz = consts.tile([128, 4], F32)
            nc.sync.dma_start(brz[:], brz_d.ap())
            bhn = consts.tile([128, 2], F32)
            nc.sync.dma_start(bhn[:], bhn_d.ap())
            bin_ = consts.tile([128, 2], F32)
            nc.sync.dma_start(bin_[:], bin_d.ap())

            h_state = [None] * G
            for t in range(KSIZE):
                for g in range(G):
                    xs = xp.tile([128, KT, NS], MM_DT, tag="x")
                    nc.sync.dma_start(xs[:], xt_d.ap()[t, g])
                    xr = xs[:]
                    hr = None if t == 0 else h_state[g][:]

                    # PSUM banks: [128, 2, NS] f32 = one 2KB bank each.
                    bank_r = ps.tile([128, 2, NS], F32, tag="r")
                    bank_z = ps.tile([128, 2, NS], F32, tag="z")
                    bank_in = ps.tile([128, 2, NS], F32, tag="in")
                    bank_hn = None if t == 0 else ps.tile([128, 2, NS], F32, tag="hn")

                    # Matmuls. W row tiles: r halves mi=0,1; z mi=2,3; n mi=4,5.
                    # Emission (= PE priority) order follows the dependency
                    # chain: r first (its sigmoid leads), then hn (feeds tmp),
                    # then z / in (consumed later).
                    def mm_accum(bank_t, mi, m, with_h):
                        col = slice(mi * 128, (mi + 1) * 128)
                        n_mm = 2 * KT if with_h else KT
                        i_mm = 0
                        for k in range(KT):
                            nc.tensor.matmul(
                                bank_t[:, m, :], wih[:, k, col], xr[:, k, :],
                                start=(i_mm == 0), stop=(i_mm == n_mm - 1),
                            )
                            i_mm += 1
                        if with_h:
                            for k in range(KT):
                                nc.tensor.matmul(
                                    bank_t[:, m, :], whh[:, k, col], hr[:, k, :],
                                    start=False, stop=(i_mm == n_mm - 1),
                                )
                                i_mm += 1

                    def mm_h_only(bank_t, mi, m):
                        col = slice(mi * 128, (mi + 1) * 128)
                        for k in range(KT):
                            nc.tensor.matmul(
                                bank_t[:, m, :], whh[:, k, col], hr[:, k, :],
                                start=(k == 0), stop=(k == KT - 1),
                            )

                    for m in range(2):
                        mm_accum(bank_r, m, m, t > 0)
                    if t > 0:
                        for m in range(2):
                            mm_h_only(bank_hn, 4 + m, m)
                    for m in range(2):
                        mm_accum(bank_z, 2 + m, m, t > 0)
                    for m in range(2):
                        mm_accum(bank_in, 4 + m, m, False)

                    # Elementwise.
                    # Separate r / z tiles so tmp's read of r never waits
                    # on the (later) z sigmoid writes.
                    r_t = work.tile([128, 2, NS], MM_DT, tag="rg")
                    z_t = work.tile([128, 2, NS], MM_DT, tag="zg")
                    for mi in range(2):  # r halves first: r leads the chain
                        nc.scalar.activation(
                            r_t[:, mi, :], bank_r[:, mi, :], AF.Sigmoid,
                            bias=brz[:, mi : mi + 1],
                        )
                    for mi in range(2):  # z halves after (consumed late)
                        nc.scalar.activation(
                            z_t[:, mi, :], bank_z[:, mi, :], AF.Sigmoid,
                            bias=brz[:, 2 + mi : 3 + mi],
                        )

                    tmp = work.tile([128, 2, NS], MM_DT, tag="tmp")
                    pren = work.tile([128, 2, NS], MM_DT, tag="pren")
                    for m in range(2):
                        if t == 0:
                            # h=0: h-side n contribution is just b_hh_n.
                            nc.vector.tensor_scalar_mul(
                                tmp[:, m, :], r_t[:, m, :], bhn[:, m : m + 1]
                            )
                        else:
                            # tmp = (psum_hn + b_hh_n) * r
                            nc.vector.scalar_tensor_tensor(
                                tmp[:, m, :], bank_hn[:, m, :], bhn[:, m : m + 1],
                                r_t[:, m, :], op0=OP.add, op1=OP.mult,
                            )
                        # pre_n = (psum_in + b_ih_n) + tmp
                        nc.vector.scalar_tensor_tensor(
                            pren[:, m, :], bank_in[:, m, :], bin_[:, m : m + 1],
                            tmp[:, m, :], op0=OP.add, op1=OP.add,
                        )

                    n_t = work.tile([128, 2, NS], MM_DT, tag="n")
                    nc.scalar.activation(n_t[:], pren[:], AF.Tanh)

                    hnew = hp.tile([128, 2, NS], MM_DT, tag="h")
                    e = work.tile([128, 2, NS], MM_DT, tag="e")
                    if t == 0:
                        # h1 = n - z*n
                        nc.vector.tensor_tensor(e[:], z_t[:], n_t[:], op=OP.mult)
                        nc.vector.tensor_tensor(hnew[:], n_t[:], e[:], op=OP.subtract)
                    else:
                        d = work.tile([128, 2, NS], MM_DT, tag="d")
                        # h = n + z*(h_prev - n)
                        nc.vector.tensor_tensor(
                            d[:], h_state[g][:], n_t[:], op=OP.subtract
                        )
                        nc.vector.tensor_tensor(e[:], z_t[:], d[:], op=OP.mult)
                        nc.vector.tensor_tensor(hnew[:], e[:], n_t[:], op=OP.add)

                    nc.sync.dma_start(out_d.ap()[t, g], hnew[:])
                    h_state[g] = hnew

    nc.compile()
    return nc


_NC_CACHE = None


def _get_nc():
    global _NC_CACHE
    if _NC_CACHE is None:
        _NC_CACHE = build_nc()
    return _NC_CACHE


def _prep_shared(W_ih, W_hh, b_ih, b_hh):
    wih_t = np.ascontiguousarray(
        W_ih.T.reshape(KT, 128, 3 * H).transpose(1, 0, 2)
    ).astype(NP_MM_DT)
    whh_t = np.ascontiguousarray(
        W_hh.T.reshape(KT, 128, 3 * H).transpose(1, 0, 2)
    ).astype(NP_MM_DT)
    bsum = b_ih + b_hh
    brz = np.ascontiguousarray(bsum[: 2 * H].reshape(4, 128).T)
    bhn = np.ascontiguousarray(b_hh[2 * H :].reshape(2, 128).T)
    bin_ = np.ascontiguousarray(b_ih[2 * H :].reshape(2, 128).T)
    return wih_t, whh_t, brz, bhn, bin_


def _prep_core_inputs(x, shared, core):
    wih_t, whh_t, brz, bhn, bin_ = shared
    xc = x[core * ROWS_PER_CORE : (core + 1) * ROWS_PER_CORE]  # [4, S, I]
    xc = xc.reshape(SEQS, KSIZE, I)
    # xt[t, g, p, k, s] = xc[g*NS+s, t, k*128+p]
    xt = np.ascontiguousarray(
        xc.reshape(G, NS, KSIZE, KT, 128).transpose(2, 0, 4, 3, 1)
    ).astype(NP_MM_DT)
    return {
        "xt": xt,
        "wih_t": wih_t,
        "whh_t": whh_t,
        "brz": brz,
        "bhn": bhn,
        "bin": bin_,
    }


def kernel(x, W_ih, W_hh, b_ih, b_hh, ksize):
    x = np.asarray(x, dtype=np.float32)
    W_ih = np.asarray(W_ih, dtype=np.float32)
    W_hh = np.asarray(W_hh, dtype=np.float32)
    b_ih = np.asarray(b_ih, dtype=np.float32)
    b_hh = np.asarray(b_hh, dtype=np.float32)
    assert int(ksize) == KSIZE and x.shape == (B, S, I)

    shared = _prep_shared(W_ih, W_hh, b_ih, b_hh)
    in_maps = [_prep_core_inputs(x, shared, c) for c in range(NCORES)]
    nc = _get_nc()
    res = run_bass_kernel_spmd(nc, in_maps, core_ids=list(range(NCORES)))

    out = np.empty((B, S, H), dtype=np.float32)
    for c in range(NCORES):
        oc = np.asarray(res.results[c]["out"]).astype(np.float32)  # [t,g,p,m,s]
        # h[seq=g*NS+s, t, hdim=m*128+p]
        hc = oc.transpose(1, 4, 0, 3, 2).reshape(SEQS, KSIZE, H)
        out[c * ROWS_PER_CORE : (c + 1) * ROWS_PER_CORE] = hc.reshape(
            ROWS_PER_CORE, S, H
        )
    return out



# revision 3
# speedup vs baseline: 1.1810x; 1.1810x over previous
"""Trainium2 Bass kernel for nn_LocalRNN (local GRU, chunked scan).

Problem: B=32, S=2048, I=H=256, ksize=16. Each ksize-chunk runs a GRU from
h0=0, so the 32*128=4096 chunks are independent length-16 GRU chains.

Sharding: data-parallel over chunks — core c gets batch rows [4c:4c+4],
i.e. 512 chains. Weights replicated.

Per-core layout ("transposed"): gate/hidden dim on partitions, chain (seq)
index on the free dim. Two seq groups (G=2 x NS=256) ping-pong so one
group's elementwise chain hides under the other group's matmuls.

Per step t, group g (PSUM banks r, z, n, h — 4 per group, 8 total):

  bank_r = W_ir x_t                + W_hr h    (+ ACT bias port at sigmoid)
  bank_z = W_iz x_t                + W_hz h    (+ ACT bias port at sigmoid)
  bank_n = b_in (K=1 bias-mm)      + W_in x_t
  bank_h = b_hn (K=1 bias-mm)      + W_hn h
  r = sigmoid(bank_r + b_rz) ; z = sigmoid(bank_z + b_rz)    (ScalarE)
  tmp  = bank_h * r          ; pren = bank_n + tmp           (DVE, full-bank)
  n = tanh(pren)                                             (ScalarE)
  h' = z*h - (z-1)*n   via e1 = z*h (off-chain), e2 = (z-1)*n

The b_in/b_hn biases are pre-added into PSUM by tiny K=1 matmuls
(lhsT = bias half [1,128], moving = ones [1,256]), which lets tmp/pren run
as single full-bank DVE tensor_tensor ops instead of four per-half
scalar_tensor_tensor ops.

Startup: ~80 junk matmuls warm the PE HAM clock gate (1.2 -> 2.4 GHz)
while the weight/x DMAs land; x is DMA'd in 4-step blocks (512KB each) so
descriptor generation stops stalling the PE mid-kernel; outputs are staged
in SBUF and written back in 4-step blocks too.

Matmul operands and SBUF elementwise tensors are fp16 (values are O(1) so
fp16 range is safe); PSUM accumulation is fp32. Host pre-transposes x /
weights into DMA-friendly contiguous blocks and inverts the output layout
at the end.
"""

import sys

for _p in ("/opt/trn_rl_repo", "/root/.axon_site"):
    if _p not in sys.path:
        sys.path.insert(0, _p)

import ml_dtypes  # noqa: F401
import numpy as np

import concourse.bass as bass  # noqa: F401
import concourse.tile as tile
from concourse import bacc, mybir
from concourse.bass_utils import run_bass_kernel_spmd

# Problem constants (hardcoded per harness contract).
B, S, I, H = 32, 2048, 256, 256
KSIZE = 16
NCORES = 8
ROWS_PER_CORE = B // NCORES            # 4 batch rows per core
CHUNKS_PER_ROW = S // KSIZE            # 128
SEQS = ROWS_PER_CORE * CHUNKS_PER_ROW  # 512 chains per core
G = 2                                  # seq groups per core
NS = SEQS // G                         # 256 seqs per group
KT = 2                                 # contraction tiles (I/128 = H/128 = 2)
TBLK = 4                               # steps per DMA block
NBLK = KSIZE // TBLK                   # 4 blocks

WARM_MMS = 80                          # junk matmuls to warm the PE clock

F32 = mybir.dt.float32
F16 = mybir.dt.float16
AF = mybir.ActivationFunctionType
OP = mybir.AluOpType

MM_DT = F16
NP_MM_DT = np.float16


def build_nc():
    nc = bacc.Bacc("TRN2", target_bir_lowering=False, debug=False)

    # Inputs (host pre-transposed, contiguous per-DMA blocks).
    # xt[g, b, p, tt, k, s] = x_shard[seq=g*NS+s, t=b*TBLK+tt, i=k*128+p]
    xt_d = nc.dram_tensor(
        "xt", [G, NBLK, 128, TBLK, KT, NS], MM_DT, kind="ExternalInput"
    )
    # wih_t[p, k, m] = W_ih[m, k*128+p]  (transposed weight, lhsT layout)
    wih_d = nc.dram_tensor("wih_t", [128, KT, 3 * H], MM_DT, kind="ExternalInput")
    whh_d = nc.dram_tensor("whh_t", [128, KT, 3 * H], MM_DT, kind="ExternalInput")
    # brz[p, mi] = (b_ih+b_hh)[mi*128+p] for mi in 0..3 (r0,r1,z0,z1)
    brz_d = nc.dram_tensor("brz", [128, 4], F32, kind="ExternalInput")
    # bias-matmul lhsT rows: [b_hn0, b_hn1, b_in0, b_in1], each 128 wide.
    bias4_d = nc.dram_tensor("bias4", [1, 512], MM_DT, kind="ExternalInput")
    # out[g, b, p, tt, m, s] = h_t[seq=g*NS+s, t=b*TBLK+tt, hdim=m*128+p]
    out_d = nc.dram_tensor(
        "out", [G, NBLK, 128, TBLK, 2, NS], MM_DT, kind="ExternalOutput"
    )

    with tile.TileContext(nc) as tc:
        with (
            tc.tile_pool(name="consts", bufs=1) as consts,
            tc.tile_pool(name="xp", bufs=4) as xp,
            tc.tile_pool(name="ps", bufs=2, space="PSUM") as ps,
            tc.tile_pool(name="work", bufs=4) as work,
            tc.tile_pool(name="ho", bufs=4) as hop,
        ):
            # --- PE warm-up: junk matmuls while DMAs land (HAM 1.2->2.4GHz).
            junk = consts.tile([128, 128], MM_DT)
            nc.vector.memset(junk[:], 0.0)
            ones = consts.tile([1, NS], MM_DT)
            nc.vector.memset(ones[:], 1.0)
            warm_ps = ps.tile([128, 2, NS], F32, tag="r")
            for _ in range(WARM_MMS):
                nc.tensor.matmul(
                    warm_ps[:, 0, :128], junk[:], junk[:],
                    start=True, stop=True, skip_group_check=True,
                )

            # --- Constants. DMA order puts first-needed data first.
            wih = consts.tile([128, KT, 3 * H], MM_DT)
            nc.sync.dma_start(wih[:], wih_d.ap())
            xblk = {}
            for g in range(G):
                xb = xp.tile([128, TBLK, KT, NS], MM_DT, tag="x")
                nc.sync.dma_start(xb[:], xt_d.ap()[g, 0])
                xblk[g] = xb
            brz = consts.tile([128, 4], F32)
            nc.sync.dma_start(brz[:], brz_d.ap())
            bias4 = consts.tile([1, 512], MM_DT)
            nc.sync.dma_start(bias4[:], bias4_d.ap())
            whh = consts.tile([128, KT, 3 * H], MM_DT)
            nc.sync.dma_start(whh[:], whh_d.ap())

            h_prev = [None] * G   # AP view of previous step's h
            ho_t = [None] * G     # current output staging tile
            gates = [None] * G    # (r_t, z_t) SBUF fp16 tiles
            n_t = [None] * G
            e1 = [None] * G

            def x_mm(bank, g, tt, mi, m, start, stop):
                # x-side gate matmuls: W_ih rows [mi*128,(mi+1)*128) x k-tiles
                col = slice(mi * 128, (mi + 1) * 128)
                for k in range(KT):
                    nc.tensor.matmul(
                        bank[:, m, :], wih[:, k, col], xblk[g][:, tt, k, :],
                        start=(start and k == 0), stop=(stop and k == KT - 1),
                    )

            def h_mm(bank, g, mi, m, stop):
                col = slice(mi * 128, (mi + 1) * 128)
                for k in range(KT):
                    nc.tensor.matmul(
                        bank[:, m, :], whh[:, k, col], h_prev[g][:, k, :],
                        start=False, stop=(stop and k == KT - 1),
                    )

            def bias_mm(bank, row, m, stop):
                # bank[:, m, :] = bias4[row*128:(row+1)*128] broadcast over seqs
                col = slice(row * 128, (row + 1) * 128)
                nc.tensor.matmul(
                    bank[:, m, :], bias4[:, col], ones[:],
                    start=True, stop=stop,
                )

            for t in range(KSIZE):
                blk, tt = divmod(t, TBLK)
                if tt == 0 and blk + 1 < NBLK:
                    # Prefetch next x block for both groups.
                    for g in range(G):
                        xb = xp.tile([128, TBLK, KT, NS], MM_DT, tag="x")
                        nc.sync.dma_start(xb[:], xt_d.ap()[g, blk + 1])
                        xblk[(g, blk + 1)] = xb
                if tt == 0 and blk > 0:
                    for g in range(G):
                        xblk[g] = xblk.pop((g, blk))

                bank_n, bank_h = {}, {}
                bank_r, bank_z = {}, {}
                # PART A: x-side + bias matmuls (independent of h'(t-1)).
                for g in range(G):
                    bank_r[g] = ps.tile([128, 2, NS], F32, tag="r")
                    bank_z[g] = ps.tile([128, 2, NS], F32, tag="z")
                    bank_n[g] = ps.tile([128, 2, NS], F32, tag="n")
                    bank_h[g] = ps.tile([128, 2, NS], F32, tag="h")
                    for m in range(2):
                        x_mm(bank_r[g], g, tt, m, m, start=True, stop=(t == 0))
                    for m in range(2):
                        x_mm(bank_z[g], g, tt, 2 + m, m, start=True,
                             stop=(t == 0))
                    for m in range(2):
                        bias_mm(bank_h[g], m, m, stop=(t == 0))
                    for m in range(2):
                        bias_mm(bank_n[g], 2 + m, m, stop=False)
                        x_mm(bank_n[g], g, tt, 4 + m, m, start=False,
                             stop=True)

                # PART B: h-side matmuls + sigmoids (r first — it leads).
                for g in range(G):
                    if t > 0:
                        for m in range(2):
                            h_mm(bank_r[g], g, m, m, stop=True)
                        for m in range(2):
                            h_mm(bank_h[g], g, 4 + m, m, stop=True)
                        for m in range(2):
                            h_mm(bank_z[g], g, 2 + m, m, stop=True)
                    r_t = work.tile([128, 2, NS], MM_DT, tag="rg")
                    z_t = work.tile([128, 2, NS], MM_DT, tag="zg")
                    for mi in range(2):
                        nc.scalar.activation(
                            r_t[:, mi, :], bank_r[g][:, mi, :], AF.Sigmoid,
                            bias=brz[:, mi : mi + 1],
                        )
                    for mi in range(2):
                        nc.scalar.activation(
                            z_t[:, mi, :], bank_z[g][:, mi, :], AF.Sigmoid,
                            bias=brz[:, 2 + mi : 3 + mi],
                        )
                    gates[g] = (r_t, z_t)

                # PART C: n-path (full-bank DVE ops) + off-chain e1.
                for g in range(G):
                    r_t, z_t = gates[g]
                    tmp = work.tile([128, 2, NS], MM_DT, tag="tmp")
                    nc.vector.tensor_tensor(
                        tmp[:], bank_h[g][:], r_t[:], op=OP.mult
                    )
                    pren = work.tile([128, 2, NS], MM_DT, tag="pren")
                    nc.vector.tensor_tensor(
                        pren[:], bank_n[g][:], tmp[:], op=OP.add
                    )
                    nt = work.tile([128, 2, NS], MM_DT, tag="n")
                    nc.scalar.activation(nt[:], pren[:], AF.Tanh)
                    n_t[g] = nt
                    if t > 0:
                        # e1 = z*h_prev — off the tanh chain, overlaps it.
                        e1g = work.tile([128, 2, NS], MM_DT, tag="e1")
                        nc.vector.tensor_tensor(
                            e1g[:], z_t[:], h_prev[g][:], op=OP.mult
                        )
                        e1[g] = e1g

                # PART D: h-update + staged output DMA.
                for g in range(G):
                    _, z_t = gates[g]
                    if tt == 0:
                        ho_t[g] = hop.tile([128, TBLK, 2, NS], MM_DT, tag="ho")
                    e2 = work.tile([128, 2, NS], MM_DT, tag="e2")
                    # e2 = (z - 1) * n
                    nc.vector.scalar_tensor_tensor(
                        e2[:], z_t[:], 1.0, n_t[g][:],
                        op0=OP.subtract, op1=OP.mult,
                    )
                    hnew = ho_t[g][:, tt, :, :]
                    if t == 0:
                        # h1 = (1-z)*n = -e2
                        nc.vector.tensor_scalar_mul(hnew, e2[:], -1.0)
                    else:
                        # h' = z*h - (z-1)*n = e1 - e2
                        nc.vector.tensor_tensor(
                            hnew, e1[g][:], e2[:], op=OP.subtract
                        )
                    h_prev[g] = hnew
                    if tt == TBLK - 1:
                        nc.sync.dma_start(out_d.ap()[g, blk], ho_t[g][:])

    nc.compile()
    return nc


_NC_CACHE = None


def _get_nc():
    global _NC_CACHE
    if _NC_CACHE is None:
        _NC_CACHE = build_nc()
    return _NC_CACHE


def _prep_shared(W_ih, W_hh, b_ih, b_hh):
    wih_t = np.ascontiguousarray(
        W_ih.T.reshape(KT, 128, 3 * H).transpose(1, 0, 2)
    ).astype(NP_MM_DT)
    whh_t = np.ascontiguousarray(
        W_hh.T.reshape(KT, 128, 3 * H).transpose(1, 0, 2)
    ).astype(NP_MM_DT)
    bsum = b_ih + b_hh
    brz = np.ascontiguousarray(bsum[: 2 * H].reshape(4, 128).T)
    bias4 = np.concatenate([b_hh[2 * H :], b_ih[2 * H :]]).reshape(1, 512)
    bias4 = np.ascontiguousarray(bias4).astype(NP_MM_DT)
    return wih_t, whh_t, brz, bias4


def _prep_core_inputs(x, shared, core):
    wih_t, whh_t, brz, bias4 = shared
    xc = x[core * ROWS_PER_CORE : (core + 1) * ROWS_PER_CORE]  # [4, S, I]
    xc = xc.reshape(SEQS, KSIZE, I)
    # xt[g, b, p, tt, k, s] = xc[g*NS+s, b*TBLK+tt, k*128+p]
    xt = np.ascontiguousarray(
        xc.reshape(G, NS, NBLK, TBLK, KT, 128).transpose(0, 2, 5, 3, 4, 1)
    ).astype(NP_MM_DT)
    return {
        "xt": xt,
        "wih_t": wih_t,
        "whh_t": whh_t,
        "brz": brz,
        "bias4": bias4,
    }


def kernel(x, W_ih, W_hh, b_ih, b_hh, ksize):
    x = np.asarray(x, dtype=np.float32)
    W_ih = np.asarray(W_ih, dtype=np.float32)
    W_hh = np.asarray(W_hh, dtype=np.float32)
    b_ih = np.asarray(b_ih, dtype=np.float32)
    b_hh = np.asarray(b_hh, dtype=np.float32)
    assert int(ksize) == KSIZE and x.shape == (B, S, I)

    shared = _prep_shared(W_ih, W_hh, b_ih, b_hh)
    in_maps = [_prep_core_inputs(x, shared, c) for c in range(NCORES)]
    nc = _get_nc()
    res = run_bass_kernel_spmd(nc, in_maps, core_ids=list(range(NCORES)))

    out = np.empty((B, S, H), dtype=np.float32)
    for c in range(NCORES):
        oc = np.asarray(res.results[c]["out"]).astype(np.float32)
        # oc[g, b, p, tt, m, s] -> h[seq=g*NS+s, t=b*TBLK+tt, hdim=m*128+p]
        hc = oc.transpose(0, 5, 1, 3, 4, 2).reshape(SEQS, KSIZE, H)
        out[c * ROWS_PER_CORE : (c + 1) * ROWS_PER_CORE] = hc.reshape(
            ROWS_PER_CORE, S, H
        )
    return out
